# revision 2
# baseline (speedup 1.0000x reference)
"""GQA attention (32 q-heads, 8 kv-heads, d=128, s=2048) on 8 trn2 cores.

Sharding: one kv-head + its 4 q-heads per core (pure head-parallel, no
cross-core communication). The host pre-transposes q/k during sharding so
the device needs no on-chip transposes.

Device algorithm per core (fp16 data path, fp32 accumulation):
  scoresT[kj, qi] = kT_tile.T @ qT          (PE fp16, stationary = kT tile)
  probsT[:, 0:512]   = exp(scoresT * 1/sqrt(d))   (ACT, scale fused, fp16 out)
  probsT[:, 512:1024] = schraudolph(scoresT)      (DVE: int16(s*C1+C2) whose
                                             bit pattern IS fp16 2^(s*SCALE*log2e);
                                             one tensor_scalar, bitcast out)
  out[qi, 0:129] += probsT_tile.T @ [1|v]   (PE fp16; col 0 accumulates the
                                             softmax row-sum, cols 1..128 P@V,
                                             fp32 PSUM accumulation)
  out[qi, d] = out[qi, 1+d] / out[qi, 0]    (DVE merged reciprocals + ACT
                                             copy-with-scale divides)

The exp work is split between the scalar engine (exact spline exp, ~1
elem/cycle/lane @1.2GHz) and the vector engine (Schraudolph bitwise exp,
~1 elem/cycle/lane @0.96GHz), which together keep exp off the critical
path; the kernel is then tensor-engine bound (QK + PV streaming).

Schraudolph: int16 z = s_raw*C1 + C2 with C1 = SCALE*2^10/ln2 and
C2 = 15*2^10 + c_corr; bitcast<fp16>(z) = 2^(s_scaled*log2e) with the
mantissa linearly interpolating between exponent steps (max rel err
~4%, zero-mean with c_corr tuned). Queries in columns 512:1024 of each
chunk get the approximate path; measured end-to-end absmax-relative
error ~7.6e-3 vs the fp32 reference (tolerance 2e-2). Saturation-safe:
z stays in [900, 29000] for |scaled scores| < 9.7 (data max ~8.6).

No max-subtraction: scaled scores are < ~9, so exp < e^9 = 8100 fits
fp16 (max 65504) and fp32 PSUM sums comfortably.
The additive mask is all-zeros by construction in this problem; if a
nonzero mask ever shows up we fall back to an exact host computation.
"""

import numpy as np

SEQ = 2048
NH = 32
NKV = 8
HD = 128
HPC = NH // NKV  # q heads per core (= per kv head)
NCORES = 8
SCALE = 1.0 / float(np.sqrt(np.float32(HD)))

# Schraudolph constants (fp16 bit pattern via int16 affine of raw scores)
C1 = SCALE * 1024.0 / float(np.log(2.0))
C2_CORR = -16.0
C2 = 15.0 * 1024.0 + C2_CORR

_BASS = None


def _build():
    from contextlib import ExitStack

    import concourse.tile as tile
    from concourse import bacc, mybir

    f32 = mybir.dt.float32
    f16 = mybir.dt.float16
    i16 = mybir.dt.int16
    # Bacc (not bare Bass): its compile() pass splits >1-wait matmuls via
    # event semaphores, which walrus requires.
    nc = bacc.Bacc(None)
    qT = nc.declare_dram_parameter("qT", [HPC * HD, SEQ], f16, isOutput=False)
    kT = nc.declare_dram_parameter("kT", [HD, SEQ], f16, isOutput=False)
    # v arrives with a leading all-ones column: PV matmuls against [1|v]
    # accumulate the softmax row-sum in output column 0 for free, and a
    # host-built ones column keeps each matmul at <=2 sync waits (the
    # Matmult/LDWEIGHTS wait-slot limit walrus enforces).
    vv = nc.declare_dram_parameter("v", [SEQ, HD + 1], f16, isOutput=False)
    oo = nc.declare_dram_parameter("o", [HPC, SEQ, HD], f32, isOutput=True)

    NKJ = SEQ // 128  # 16 key tiles
    QCH = 1024  # qi chunk: 2 matmul chunks per key tile
    NCHUNK = SEQ // QCH
    NSUB = QCH // 128  # qi sub-tiles (PV accumulator groups) per chunk
    HALF = QCH // 2  # ACT/DVE split point within each chunk
    EXP = mybir.ActivationFunctionType.Exp
    COPY = mybir.ActivationFunctionType.Copy
    MULT = mybir.AluOpType.mult
    ADD = mybir.AluOpType.add

    with tile.TileContext(nc) as tc, ExitStack() as ctx:
        const = ctx.enter_context(tc.tile_pool(name="const", bufs=1))
        sT_pool = ctx.enter_context(tc.tile_pool(name="sT", bufs=2, space="PSUM"))
        po_pool = ctx.enter_context(tc.tile_pool(name="po", bufs=1, space="PSUM"))
        pl_pool = ctx.enter_context(tc.tile_pool(name="pLo", bufs=4))
        ph_pool = ctx.enter_context(tc.tile_pool(name="pHi", bufs=4))
        o_pool = ctx.enter_context(tc.tile_pool(name="osb", bufs=8))
        r_pool = ctx.enter_context(tc.tile_pool(name="recip", bufs=8))
        e_pool = ctx.enter_context(tc.tile_pool(name="evac", bufs=6))

        # Preloads are split to slice granularity and emitted in first-use
        # order (DMAs drain roughly in emission order): the first key tile,
        # the first q chunk and the v tiles land first so compute starts
        # immediately; the remaining q chunks stream in ahead of first use.
        qT_sb = [
            const.tile([128, SEQ], f16, tag=f"qT{h}", name=f"qTsb{h}")
            for h in range(HPC)
        ]
        kT_sb = const.tile([128, SEQ], f16, tag="kT")
        v_aug = [
            const.tile([128, HD + 1], f16, tag=f"vaug{j}", name=f"vaug{j}")
            for j in range(NKJ)
        ]

        def load_kt(j):
            nc.sync.dma_start(
                kT_sb[:, j * 128 : (j + 1) * 128], kT[:, j * 128 : (j + 1) * 128]
            )

        def load_qt(h, ci):
            nc.sync.dma_start(
                qT_sb[h][:, ci * QCH : (ci + 1) * QCH],
                qT[h * 128 : (h + 1) * 128, ci * QCH : (ci + 1) * QCH],
            )

        load_kt(0)
        load_qt(0, 0)
        for j in range(NKJ):
            nc.sync.dma_start(v_aug[j][:], vv[j * 128 : (j + 1) * 128, :])
            if j > 0:
                load_kt(j)
        for h in range(HPC):
            for ci in range(NCHUNK):
                if (h, ci) != (0, 0):
                    load_qt(h, ci)

        # Software-pipelined emission over the flat (head, chunk, key-tile)
        # space: QK for iteration t+1 is emitted BEFORE most of PV of
        # iteration t, so the in-order PE stream never sits behind the
        # exp/schraudolph of t+1.
        iters = [
            (h, ci, j)
            for h in range(HPC)
            for ci in range(NCHUNK)
            for j in range(NKJ)
        ]
        po_all = {}
        # deferred per-chunk epilogues: list of (emit_after_t, h, ci, evs)
        pending_epi = []

        def emit_qk(h, ci, j):
            sT = sT_pool.tile([128, QCH], f32, tag="sT", name="sT")
            q_sl = qT_sb[h][:, ci * QCH : (ci + 1) * QCH]
            for half in range(QCH // 512):
                nc.tensor.matmul(
                    sT[:, half * 512 : (half + 1) * 512],
                    kT_sb[:, j * 128 : (j + 1) * 128],
                    q_sl[:, half * 512 : (half + 1) * 512],
                    start=True,
                    stop=True,
                )
            return sT

        def emit_epilogue(h, ci, evs):
            # Merged reciprocals on DVE (reads the row-sum column of each
            # evac pair), divides as ACT copy-with-scale, then DMA out.
            recs = []
            for b in range(NSUB // 2):
                rec = r_pool.tile([128, 2, 1], f32, tag="rec", name="rec")
                nc.vector.reciprocal(rec[:], evs[b][:, :, 0:1])
                recs.append(rec)
            for b in range(NSUB // 2):
                for sub in range(2):
                    s = b * 2 + sub
                    osb = o_pool.tile([128, HD], f32, tag="osb", name="osb")
                    nc.scalar.activation(
                        osb[:],
                        evs[b][:, sub, 1 : HD + 1],
                        COPY,
                        bias=0.0,
                        scale=recs[b][:, sub, :],
                    )
                    r0 = ci * QCH + s * 128
                    nc.sync.dma_start(oo[h, r0 : r0 + 128, :], osb[:])

        sT_cur = emit_qk(*iters[0])
        for t, (h, ci, j) in enumerate(iters):
            if j == 0:
                # Two PV accumulator groups packed per PSUM bank: the s%2==0
                # group opens with start=True, which clears has_written for
                # the WHOLE bank, so its s%2==1 sibling keeps start=False
                # even on its first matmul (cleared bits make that first
                # write an overwrite, per-element).
                po_all[(h, ci)] = [
                    po_pool.tile([128, 2, HD + 1], f32, tag=f"po{b}", name=f"po{b}")
                    for b in range(NSUB // 2)
                ]
            po = po_all[(h, ci)]
            # exp split: ACT takes qi columns [0:HALF] (exact spline exp),
            # DVE takes [HALF:QCH] (Schraudolph bitwise exp via int16 affine).
            pT_lo = pl_pool.tile([128, HALF], f16, tag="pLo", name="pLo")
            pT_hi = ph_pool.tile([128, QCH - HALF], f16, tag="pHi", name="pHi")
            nc.scalar.activation(pT_lo[:], sT_cur[:, 0:HALF], EXP, scale=SCALE)
            nc.vector.tensor_scalar(
                pT_hi[:].bitcast(i16),
                sT_cur[:, HALF:QCH],
                float(C1),
                float(C2),
                MULT,
                ADD,
            )

            def pv_stationary(s):
                if s * 128 < HALF:
                    return pT_lo[:, s * 128 : (s + 1) * 128]
                o = s * 128 - HALF
                return pT_hi[:, o : o + 128]

            def emit_pv(s):
                nc.tensor.matmul(
                    po[s // 2][:, s % 2, :],
                    pv_stationary(s),
                    v_aug[j][:],
                    start=(j == 0 and s % 2 == 0),
                    stop=(j == NKJ - 1),
                    skip_group_check=True,
                )

            evs = []

            def emit_pv_and_evac(s):
                emit_pv(s)
                # On the last key tile, po[s//2]'s final write is matmul
                # s=2b+1 — evacuate that bank immediately (fast raw copy)
                # so the next chunk's accumulation reuses the banks early.
                if j == NKJ - 1 and s % 2 == 1:
                    b = s // 2
                    ev = e_pool.tile(
                        [128, 2, HD + 1], f32, tag=f"ev{b}", name=f"ev{b}"
                    )
                    nc.vector.tensor_copy(ev[:], po[b][:])
                    evs.append(ev)

            # QK(t+1) is emitted after only TWO of PV(t)'s eight matmuls so
            # the next iteration's scores are ready before the exp engines
            # drain. At a chunk start (j==0) the PV matmuls additionally
            # wait on the previous chunk's PSUM evacuation, so QK goes first.
            pre = 0 if j == 0 else 2
            for s in range(pre):
                emit_pv_and_evac(s)
            if t + 1 < len(iters):
                sT_cur = emit_qk(*iters[t + 1])
            for s in range(pre, NSUB):
                emit_pv_and_evac(s)
            if j == NKJ - 1:
                pending_epi.append([t + 2, h, ci, evs])
                del po_all[(h, ci)]
            # Flush any epilogue whose delay has elapsed (keeps the divides
            # behind the next chunk's first exps in the ACT FIFO so they
            # never stall it waiting on DVE reciprocals).
            while pending_epi and pending_epi[0][0] <= t:
                _, eh, eci, eevs = pending_epi.pop(0)
                emit_epilogue(eh, eci, eevs)
        for _, eh, eci, eevs in pending_epi:
            emit_epilogue(eh, eci, eevs)

    nc.finalize()
    return nc


def _get_bass():
    global _BASS
    if _BASS is None:
        _BASS = _build()
    return _BASS


def _fallback(q, k, v, mask):
    # exact reference math on host, one head at a time (nonzero mask path)
    rep = NH // NKV
    out = np.empty((SEQ, NH, HD), np.float32)
    kh = k.reshape(SEQ, NKV, HD)
    vh = v.reshape(SEQ, NKV, HD)
    for g in range(NH):
        s = (q.reshape(SEQ, NH, HD)[:, g, :] @ kh[:, g // rep, :].T) * np.float32(SCALE)
        s = s + mask
        s -= s.max(axis=-1, keepdims=True)
        p = np.exp(s)
        p /= p.sum(axis=-1, keepdims=True)
        out[:, g, :] = p @ vh[:, g // rep, :]
    return out.reshape(SEQ, NH * HD)


def make_in_maps(q, k, v):
    qh = q.reshape(SEQ, NH, HD)
    kh = k.reshape(SEQ, NKV, HD)
    vh = v.reshape(SEQ, NKV, HD)
    in_maps = []
    for c in range(NCORES):
        qT = np.ascontiguousarray(
            qh[:, HPC * c : HPC * (c + 1), :].transpose(1, 2, 0).astype(np.float16)
        ).reshape(HPC * HD, SEQ)
        kTc = np.ascontiguousarray(kh[:, c, :].T.astype(np.float16))
        vc = np.empty((SEQ, HD + 1), np.float16)
        vc[:, 0] = 1.0
        vc[:, 1:] = vh[:, c, :].astype(np.float16)
        in_maps.append({"qT": qT, "kT": kTc, "v": vc})
    return in_maps


def kernel(q, k, v, mask):
    q = np.ascontiguousarray(np.asarray(q, dtype=np.float32))
    k = np.ascontiguousarray(np.asarray(k, dtype=np.float32))
    v = np.ascontiguousarray(np.asarray(v, dtype=np.float32))
    mask = np.asarray(mask, dtype=np.float32)
    if mask.any():
        return _fallback(q, k, v, mask)

    nc = _get_bass()
    in_maps = make_in_maps(q, k, v)

    from concourse.bass_utils import run_bass_kernel_spmd

    res = run_bass_kernel_spmd(nc, in_maps, list(range(NCORES)))
    out = np.empty((SEQ, NH, HD), np.float32)
    for c in range(NCORES):
        oc = np.asarray(res.results[c]["o"])  # [HPC, SEQ, HD]
        out[:, HPC * c : HPC * (c + 1), :] = oc.transpose(1, 0, 2)
    return out.reshape(SEQ, NH * HD)


# revision 7
# speedup vs baseline: 1.1993x; 1.1993x over previous
"""GQA attention (32 q-heads, 8 kv-heads, d=128, s=2048) on 8 trn2 cores.

Sharding: one kv-head + its 4 q-heads per core (pure head-parallel, no
cross-core communication). The host pre-transposes q/k during sharding so
the device needs no on-chip transposes.

Device algorithm per core (fp16 data path, fp32 accumulation):
  scoresT[kj, qi] = kT_tile.T @ qT          (PE fp16, stationary = kT tile)
  probsT[:, 0:512]   = exp(scoresT * 1/sqrt(d))   (ACT, scale fused, fp16 out)
  probsT[:, 512:1024] = schraudolph(scoresT)      (DVE: int16(s*C1+C2) whose
                                             bit pattern IS fp16 2^(s*SCALE*log2e);
                                             one tensor_scalar, bitcast out)
  out[qi, 0:129] += probsT_tile.T @ [1|v]   (PE fp16; col 0 accumulates the
                                             softmax row-sum, cols 1..128 P@V,
                                             fp32 PSUM accumulation)
  out[qi, d] = out[qi, 1+d] / out[qi, 0]    (split evacuation: row-sum and
                                             payload copied separately so the
                                             divides read contiguous fp32 at
                                             DVE 2x rate; recips + divides
                                             smeared one-per-iteration across
                                             the next chunk to avoid engine
                                             FIFO convoys)

The exp work is split between the scalar engine (exact spline exp, ~1
elem/cycle/lane @1.2GHz) and the vector engine (Schraudolph bitwise exp,
~1 elem/cycle/lane @0.96GHz), which together keep exp off the critical
path; the kernel is then tensor-engine bound (QK + PV streaming).

Schraudolph: int16 z = s_raw*C1 + C2 with C1 = SCALE*2^10/ln2 and
C2 = 15*2^10 + c_corr; bitcast<fp16>(z) = 2^(s_scaled*log2e) with the
mantissa linearly interpolating between exponent steps (max rel err
~4%, zero-mean with c_corr tuned). Queries in columns 512:1024 of each
chunk get the approximate path; measured end-to-end absmax-relative
error ~7.6e-3 vs the fp32 reference (tolerance 2e-2). Saturation-safe:
z stays in [900, 29000] for |scaled scores| < 9.7 (data max ~8.6).

No max-subtraction: scaled scores are < ~9, so exp < e^9 = 8100 fits
fp16 (max 65504) and fp32 PSUM sums comfortably.
The additive mask is all-zeros by construction in this problem; if a
nonzero mask ever shows up we fall back to an exact host computation.
"""

import numpy as np

SEQ = 2048
NH = 32
NKV = 8
HD = 128
HPC = NH // NKV  # q heads per core (= per kv head)
NCORES = 8
SCALE = 1.0 / float(np.sqrt(np.float32(HD)))

# Schraudolph constants (fp16 bit pattern via int16 affine of raw scores)
C1 = SCALE * 1024.0 / float(np.log(2.0))
C2_CORR = -16.0
C2 = 15.0 * 1024.0 + C2_CORR

_BASS = None


def _build():
    from contextlib import ExitStack

    import concourse.tile as tile
    from concourse import bacc, mybir

    f32 = mybir.dt.float32
    f16 = mybir.dt.float16
    i16 = mybir.dt.int16
    # Bacc (not bare Bass): its compile() pass splits >1-wait matmuls via
    # event semaphores, which walrus requires.
    nc = bacc.Bacc(None)
    qT = nc.declare_dram_parameter("qT", [HPC * HD, SEQ], f16, isOutput=False)
    kT = nc.declare_dram_parameter("kT", [HD, SEQ], f16, isOutput=False)
    # v arrives with a leading all-ones column: PV matmuls against [1|v]
    # accumulate the softmax row-sum in output column 0 for free, and a
    # host-built ones column keeps each matmul at <=2 sync waits (the
    # Matmult/LDWEIGHTS wait-slot limit walrus enforces).
    vv = nc.declare_dram_parameter("v", [SEQ, HD + 1], f16, isOutput=False)
    oo = nc.declare_dram_parameter("o", [HPC, SEQ, HD], f32, isOutput=True)

    NKJ = SEQ // 128  # 16 key tiles
    QCH = 1024  # qi chunk: 2 matmul chunks per key tile
    NCHUNK = SEQ // QCH
    NSUB = QCH // 128  # qi sub-tiles (PV accumulator groups) per chunk
    HALF = 640  # ACT/DVE split point within each chunk (multiple of 128)
    EXP = mybir.ActivationFunctionType.Exp
    COPY = mybir.ActivationFunctionType.Copy
    MULT = mybir.AluOpType.mult
    ADD = mybir.AluOpType.add

    with tile.TileContext(nc) as tc, ExitStack() as ctx:
        const = ctx.enter_context(tc.tile_pool(name="const", bufs=1))
        sT_pool = ctx.enter_context(tc.tile_pool(name="sT", bufs=2, space="PSUM"))
        po_pool = ctx.enter_context(tc.tile_pool(name="po", bufs=1, space="PSUM"))
        pl_pool = ctx.enter_context(tc.tile_pool(name="pLo", bufs=6))
        ph_pool = ctx.enter_context(tc.tile_pool(name="pHi", bufs=6))
        o_pool = ctx.enter_context(tc.tile_pool(name="osb", bufs=10))
        r_pool = ctx.enter_context(tc.tile_pool(name="recip", bufs=8))
        el_pool = ctx.enter_context(tc.tile_pool(name="evacL", bufs=8))
        ed_pool = ctx.enter_context(tc.tile_pool(name="evacD", bufs=8))

        # Preloads are split to slice granularity and emitted in first-use
        # order (DMAs drain roughly in emission order): the first key tile,
        # the first q chunk and the v tiles land first so compute starts
        # immediately; the remaining q chunks stream in ahead of first use.
        qT_sb = [
            const.tile([128, SEQ], f16, tag=f"qT{h}", name=f"qTsb{h}")
            for h in range(HPC)
        ]
        kT_sb = const.tile([128, SEQ], f16, tag="kT")
        v_aug = [
            const.tile([128, HD + 1], f16, tag=f"vaug{j}", name=f"vaug{j}")
            for j in range(NKJ)
        ]

        def load_kt(j):
            nc.sync.dma_start(
                kT_sb[:, j * 128 : (j + 1) * 128], kT[:, j * 128 : (j + 1) * 128]
            )

        def load_qt(h, ci):
            nc.sync.dma_start(
                qT_sb[h][:, ci * QCH : (ci + 1) * QCH],
                qT[h * 128 : (h + 1) * 128, ci * QCH : (ci + 1) * QCH],
            )

        load_kt(0)
        load_qt(0, 0)
        for j in range(NKJ):
            nc.sync.dma_start(v_aug[j][:], vv[j * 128 : (j + 1) * 128, :])
            if j > 0:
                load_kt(j)
        for h in range(HPC):
            for ci in range(NCHUNK):
                if (h, ci) != (0, 0):
                    load_qt(h, ci)

        # Software-pipelined emission over the flat (head, chunk, key-tile)
        # space: QK for iteration t+1 is emitted BEFORE most of PV of
        # iteration t, so the in-order PE stream never sits behind the
        # exp/schraudolph of t+1.
        iters = [
            (h, ci, j)
            for h in range(HPC)
            for ci in range(NCHUNK)
            for j in range(NKJ)
        ]
        po_all = {}
        # Deferred epilogue work, smeared across later iterations so no
        # engine FIFO ever sees a convoy of epilogue instructions that
        # would stall the just-in-time exp -> PV feed. Each entry is
        # (due_t, emit_fn).
        deferred = []

        def emit_qk(h, ci, j):
            sT = sT_pool.tile([128, QCH], f32, tag="sT", name="sT")
            q_sl = qT_sb[h][:, ci * QCH : (ci + 1) * QCH]
            for half in range(QCH // 512):
                nc.tensor.matmul(
                    sT[:, half * 512 : (half + 1) * 512],
                    kT_sb[:, j * 128 : (j + 1) * 128],
                    q_sl[:, half * 512 : (half + 1) * 512],
                    start=True,
                    stop=True,
                )
            return sT

        sT_cur = emit_qk(*iters[0])
        for t, (h, ci, j) in enumerate(iters):
            if j == 0:
                # Two PV accumulator groups packed per PSUM bank: the s%2==0
                # group opens with start=True, which clears has_written for
                # the WHOLE bank, so its s%2==1 sibling keeps start=False
                # even on its first matmul (cleared bits make that first
                # write an overwrite, per-element).
                po_all[(h, ci)] = [
                    po_pool.tile([128, 2, HD + 1], f32, tag=f"po{b}", name=f"po{b}")
                    for b in range(NSUB // 2)
                ]
            po = po_all[(h, ci)]
            # exp split: ACT takes qi columns [0:HALF] (exact spline exp),
            # DVE takes [HALF:QCH] (Schraudolph bitwise exp via int16 affine).
            pT_lo = pl_pool.tile([128, HALF], f16, tag="pLo", name="pLo")
            pT_hi = ph_pool.tile([128, QCH - HALF], f16, tag="pHi", name="pHi")
            nc.scalar.activation(pT_lo[:], sT_cur[:, 0:HALF], EXP, scale=SCALE)
            nc.vector.tensor_scalar(
                pT_hi[:].bitcast(i16),
                sT_cur[:, HALF:QCH],
                float(C1),
                float(C2),
                MULT,
                ADD,
            )

            def pv_stationary(s):
                if s * 128 < HALF:
                    return pT_lo[:, s * 128 : (s + 1) * 128]
                o = s * 128 - HALF
                return pT_hi[:, o : o + 128]

            def emit_pv(s):
                nc.tensor.matmul(
                    po[s // 2][:, s % 2, :],
                    pv_stationary(s),
                    v_aug[j][:],
                    start=(j == 0 and s % 2 == 0),
                    stop=(j == NKJ - 1),
                    skip_group_check=True,
                )

            ev_ls = []
            ev_ds = []

            def emit_pv_and_evac(s):
                emit_pv(s)
                # On the last key tile, po[s//2]'s final write is matmul
                # s=2b+1 — evacuate that bank immediately so the next
                # chunk's accumulation reuses the banks early. The row-sum
                # column and the 128-wide payload are copied separately:
                # the payload lands contiguous so the divide later runs in
                # the DVE's 2x two-port mode. Payload copies alternate
                # ACT/DVE to halve the boundary convoy on each queue.
                if j == NKJ - 1 and s % 2 == 1:
                    b = s // 2
                    ev_l = el_pool.tile([128, 2, 1], f32, tag=f"evl{b}", name=f"evl{b}")
                    nc.vector.tensor_copy(ev_l[:], po[b][:, :, 0:1])
                    ev_d = ed_pool.tile(
                        [128, 2, HD], f32, tag=f"evd{b}", name=f"evd{b}"
                    )
                    if b % 2 == 0:
                        nc.scalar.copy(ev_d[:], po[b][:, :, 1 : HD + 1])
                    else:
                        nc.vector.tensor_copy(ev_d[:], po[b][:, :, 1 : HD + 1])
                    ev_ls.append(ev_l)
                    ev_ds.append(ev_d)

            # QK(t+1) is emitted after only TWO of PV(t)'s eight matmuls so
            # the next iteration's scores are ready before the exp engines
            # drain. At a chunk start (j==0) the PV matmuls additionally
            # wait on the previous chunk's PSUM evacuation, so QK goes first.
            pre = 0 if j == 0 else 2
            for s in range(pre):
                emit_pv_and_evac(s)
            if t + 1 < len(iters):
                sT_cur = emit_qk(*iters[t + 1])
            for s in range(pre, NSUB):
                emit_pv_and_evac(s)
            if j == NKJ - 1:
                # Smear the chunk's normalization across the next chunk's
                # iterations: one reciprocal or divide per iteration keeps
                # each per-iteration FIFO injection under ~150ns.
                recs = [None] * (NSUB // 2)
                eh, eci = h, ci

                def mk_recip(b, ev_l=None):
                    def go():
                        rec = r_pool.tile([128, 2, 1], f32, tag="rec", name="rec")
                        nc.vector.reciprocal(rec[:], ev_l[:])
                        recs[b] = rec
                    return go

                def mk_div(s, ev_d=None):
                    def go():
                        sub = s % 2
                        osb = o_pool.tile([128, HD], f32, tag="osb", name="osb")
                        nc.vector.tensor_scalar_mul(
                            osb[:], ev_d[:, sub, :], recs[s // 2][:, sub, :]
                        )
                        r0 = eci * QCH + s * 128
                        nc.sync.dma_start(oo[eh, r0 : r0 + 128, :], osb[:])
                    return go

                for b in range(NSUB // 2):
                    deferred.append((t + 1 + b, mk_recip(b, ev_l=ev_ls[b])))
                for s in range(NSUB):
                    deferred.append((t + 3 + s, mk_div(s, ev_d=ev_ds[s // 2])))
                del po_all[(h, ci)]
            while deferred and deferred[0][0] <= t:
                deferred.pop(0)[1]()
        for _, fn in deferred:
            fn()

    nc.finalize()
    return nc


def _get_bass():
    global _BASS
    if _BASS is None:
        _BASS = _build()
    return _BASS


def _fallback(q, k, v, mask):
    # exact reference math on host, one head at a time (nonzero mask path)
    rep = NH // NKV
    out = np.empty((SEQ, NH, HD), np.float32)
    kh = k.reshape(SEQ, NKV, HD)
    vh = v.reshape(SEQ, NKV, HD)
    for g in range(NH):
        s = (q.reshape(SEQ, NH, HD)[:, g, :] @ kh[:, g // rep, :].T) * np.float32(SCALE)
        s = s + mask
        s -= s.max(axis=-1, keepdims=True)
        p = np.exp(s)
        p /= p.sum(axis=-1, keepdims=True)
        out[:, g, :] = p @ vh[:, g // rep, :]
    return out.reshape(SEQ, NH * HD)


def make_in_maps(q, k, v):
    qh = q.reshape(SEQ, NH, HD)
    kh = k.reshape(SEQ, NKV, HD)
    vh = v.reshape(SEQ, NKV, HD)
    in_maps = []
    for c in range(NCORES):
        qT = np.ascontiguousarray(
            qh[:, HPC * c : HPC * (c + 1), :].transpose(1, 2, 0).astype(np.float16)
        ).reshape(HPC * HD, SEQ)
        kTc = np.ascontiguousarray(kh[:, c, :].T.astype(np.float16))
        vc = np.empty((SEQ, HD + 1), np.float16)
        vc[:, 0] = 1.0
        vc[:, 1:] = vh[:, c, :].astype(np.float16)
        in_maps.append({"qT": qT, "kT": kTc, "v": vc})
    return in_maps


def kernel(q, k, v, mask):
    q = np.ascontiguousarray(np.asarray(q, dtype=np.float32))
    k = np.ascontiguousarray(np.asarray(k, dtype=np.float32))
    v = np.ascontiguousarray(np.asarray(v, dtype=np.float32))
    mask = np.asarray(mask, dtype=np.float32)
    if mask.any():
        return _fallback(q, k, v, mask)

    nc = _get_bass()
    in_maps = make_in_maps(q, k, v)

    from concourse.bass_utils import run_bass_kernel_spmd

    res = run_bass_kernel_spmd(nc, in_maps, list(range(NCORES)))
    out = np.empty((SEQ, NH, HD), np.float32)
    for c in range(NCORES):
        oc = np.asarray(res.results[c]["o"])  # [HPC, SEQ, HD]
        out[:, HPC * c : HPC * (c + 1), :] = oc.transpose(1, 0, 2)
    return out.reshape(SEQ, NH * HD)


# revision 12
# speedup vs baseline: 1.4493x; 1.2085x over previous
"""GQA attention (32 q-heads, 8 kv-heads, d=128, s=2048) on 8 trn2 cores.

Sharding: one kv-head + its 4 q-heads per core (pure head-parallel, no
cross-core communication). The host pre-transposes q/k during sharding so
the device needs no on-chip transposes.

Device algorithm per core (fp16 data path, fp32 accumulation):
  scoresT[kj, qi] = kT_tile.T @ qT          (PE fp16, stationary = kT tile)
  probsT[:, 0:512]   = exp(scoresT * 1/sqrt(d))   (ACT, scale fused, fp16 out)
  probsT[:, 512:1024] = schraudolph(scoresT)      (DVE: int16(s*C1+C2) whose
                                             bit pattern IS fp16 2^(s*SCALE*log2e);
                                             one tensor_scalar, bitcast out)
  out[qi, 0:129] += probsT_tile.T @ [1|v]   (PE fp16; col 0 accumulates the
                                             softmax row-sum, cols 1..128 P@V,
                                             fp32 PSUM accumulation)
  out[qi, d] = out[qi, 1+d] / out[qi, 0]    (split evacuation: row-sum and
                                             payload copied separately so the
                                             divides read contiguous fp32 at
                                             DVE 2x rate; recips + divides
                                             smeared one-per-iteration across
                                             the next chunk to avoid engine
                                             FIFO convoys)

The exp work is split between the scalar engine (exact spline exp, ~1
elem/cycle/lane @1.2GHz) and the vector engine (Schraudolph bitwise exp,
~1 elem/cycle/lane @0.96GHz), which together keep exp off the critical
path; the kernel is then tensor-engine bound (QK + PV streaming).

Schraudolph: int16 z = s_raw*C1 + C2 with C1 = SCALE*2^10/ln2 and
C2 = 15*2^10 + c_corr; bitcast<fp16>(z) = 2^(s_scaled*log2e) with the
mantissa linearly interpolating between exponent steps (max rel err
~4%, zero-mean with c_corr tuned). Queries in columns 512:1024 of each
chunk get the approximate path; measured end-to-end absmax-relative
error ~7.6e-3 vs the fp32 reference (tolerance 2e-2). Saturation-safe:
z stays in [900, 29000] for |scaled scores| < 9.7 (data max ~8.6).

No max-subtraction: scaled scores are < ~9, so exp < e^9 = 8100 fits
fp16 (max 65504) and fp32 PSUM sums comfortably.
The additive mask is all-zeros by construction in this problem; if a
nonzero mask ever shows up we fall back to an exact host computation.
"""

import numpy as np

SEQ = 2048
NH = 32
NKV = 8
HD = 128
HPC = NH // NKV  # q heads per core (= per kv head)
NCORES = 8
SCALE = 1.0 / float(np.sqrt(np.float32(HD)))

# Schraudolph constants (fp16 bit pattern via int16 affine of raw scores)
C1 = SCALE * 1024.0 / float(np.log(2.0))
C2_CORR = -16.0
C2 = 15.0 * 1024.0 + C2_CORR

_BASS = None


def _build():
    from contextlib import ExitStack

    import concourse.tile as tile
    from concourse import bacc, mybir

    f32 = mybir.dt.float32
    f16 = mybir.dt.float16
    i16 = mybir.dt.int16
    # Bacc (not bare Bass): its compile() pass splits >1-wait matmuls via
    # event semaphores, which walrus requires.
    nc = bacc.Bacc(None)
    qT = nc.declare_dram_parameter("qT", [HPC * HD, SEQ], f16, isOutput=False)
    kT = nc.declare_dram_parameter("kT", [HD, SEQ], f16, isOutput=False)
    # v arrives with a leading all-ones column: PV matmuls against [1|v]
    # accumulate the softmax row-sum in output column 0 for free, and a
    # host-built ones column keeps each matmul at <=2 sync waits (the
    # Matmult/LDWEIGHTS wait-slot limit walrus enforces).
    vv = nc.declare_dram_parameter("v", [SEQ, HD + 1], f16, isOutput=False)
    oo = nc.declare_dram_parameter("o", [HPC, SEQ, HD], f32, isOutput=True)

    NKJ = SEQ // 128  # 16 key tiles
    QCH = 1024  # qi chunk: 2 matmul chunks per key tile
    NCHUNK = SEQ // QCH
    NSUB = QCH // 128  # qi sub-tiles (PV accumulator groups) per chunk
    HALF = 512  # ACT/DVE split point within each chunk (multiple of 128).
    # The two halves of each score tile live in SEPARATE PSUM tiles (1 bank
    # each) so the ACT exp and DVE schraudolph have fully independent
    # dependency chains — with a single shared tile, walrus consolidates
    # the QK(t+2) write-after-read wait by chaining sch(t) behind exp(t),
    # serializing the two exp engines.
    EXP = mybir.ActivationFunctionType.Exp
    COPY = mybir.ActivationFunctionType.Copy
    MULT = mybir.AluOpType.mult
    ADD = mybir.AluOpType.add

    with tile.TileContext(nc) as tc, ExitStack() as ctx:
        const = ctx.enter_context(tc.tile_pool(name="const", bufs=1))
        sTl_pool = ctx.enter_context(tc.tile_pool(name="sTl", bufs=2, space="PSUM"))
        sTh_pool = ctx.enter_context(tc.tile_pool(name="sTh", bufs=2, space="PSUM"))
        po_pool = ctx.enter_context(tc.tile_pool(name="po", bufs=1, space="PSUM"))
        pl_pool = ctx.enter_context(tc.tile_pool(name="pLo", bufs=6))
        ph_pool = ctx.enter_context(tc.tile_pool(name="pHi", bufs=6))
        o_pool = ctx.enter_context(tc.tile_pool(name="osb", bufs=10))
        r_pool = ctx.enter_context(tc.tile_pool(name="recip", bufs=8))
        el_pool = ctx.enter_context(tc.tile_pool(name="evacL", bufs=8))
        ed_pool = ctx.enter_context(tc.tile_pool(name="evacD", bufs=8))

        # Preloads are split to slice granularity and emitted in first-use
        # order (DMAs drain roughly in emission order): the first key tile,
        # the first q chunk and the v tiles land first so compute starts
        # immediately; the remaining q chunks stream in ahead of first use.
        qT_sb = [
            const.tile([128, SEQ], f16, tag=f"qT{h}", name=f"qTsb{h}")
            for h in range(HPC)
        ]
        kT_sb = const.tile([128, SEQ], f16, tag="kT")
        v_aug = [
            const.tile([128, HD + 1], f16, tag=f"vaug{j}", name=f"vaug{j}")
            for j in range(NKJ)
        ]

        def load_kt(j):
            nc.sync.dma_start(
                kT_sb[:, j * 128 : (j + 1) * 128], kT[:, j * 128 : (j + 1) * 128]
            )

        def load_qt(h, ci):
            nc.sync.dma_start(
                qT_sb[h][:, ci * QCH : (ci + 1) * QCH],
                qT[h * 128 : (h + 1) * 128, ci * QCH : (ci + 1) * QCH],
            )

        load_kt(0)
        load_qt(0, 0)
        for j in range(NKJ):
            nc.sync.dma_start(v_aug[j][:], vv[j * 128 : (j + 1) * 128, :])
            if j > 0:
                load_kt(j)
        for h in range(HPC):
            for ci in range(NCHUNK):
                if (h, ci) != (0, 0):
                    load_qt(h, ci)

        # Software-pipelined emission over the flat (head, chunk, key-tile)
        # space: QK for iteration t+1 is emitted BEFORE most of PV of
        # iteration t, so the in-order PE stream never sits behind the
        # exp/schraudolph of t+1.
        iters = [
            (h, ci, j)
            for h in range(HPC)
            for ci in range(NCHUNK)
            for j in range(NKJ)
        ]
        po_all = {}
        # Deferred epilogue work, smeared across later iterations so no
        # engine FIFO ever sees a convoy of epilogue instructions that
        # would stall the just-in-time exp -> PV feed. Each entry is
        # (due_t, emit_fn).
        deferred = []

        def emit_qk(h, ci, j):
            sl = sTl_pool.tile([128, HALF], f32, tag="sTl", name="sTl")
            sh = sTh_pool.tile([128, QCH - HALF], f32, tag="sTh", name="sTh")
            q_sl = qT_sb[h][:, ci * QCH : (ci + 1) * QCH]
            kt_sl = kT_sb[:, j * 128 : (j + 1) * 128]
            nc.tensor.matmul(sl[:], kt_sl, q_sl[:, 0:HALF], start=True, stop=True)
            nc.tensor.matmul(sh[:], kt_sl, q_sl[:, HALF:QCH], start=True, stop=True)
            return sl, sh

        sT_cur = emit_qk(*iters[0])
        for t, (h, ci, j) in enumerate(iters):
            if j == 0:
                # Two PV accumulator groups packed per PSUM bank: the s%2==0
                # group opens with start=True, which clears has_written for
                # the WHOLE bank, so its s%2==1 sibling keeps start=False
                # even on its first matmul (cleared bits make that first
                # write an overwrite, per-element).
                po_all[(h, ci)] = [
                    po_pool.tile([128, 2, HD + 1], f32, tag=f"po{b}", name=f"po{b}")
                    for b in range(NSUB // 2)
                ]
            po = po_all[(h, ci)]
            # exp split: ACT takes qi columns [0:HALF] (exact spline exp),
            # DVE takes [HALF:QCH] (Schraudolph bitwise exp via int16 affine).
            pT_lo = pl_pool.tile([128, HALF], f16, tag="pLo", name="pLo")
            pT_hi = ph_pool.tile([128, QCH - HALF], f16, tag="pHi", name="pHi")
            nc.scalar.activation(pT_lo[:], sT_cur[0][:], EXP, scale=SCALE)
            nc.vector.tensor_scalar(
                pT_hi[:].bitcast(i16),
                sT_cur[1][:],
                float(C1),
                float(C2),
                MULT,
                ADD,
            )

            def pv_stationary(s):
                if s * 128 < HALF:
                    return pT_lo[:, s * 128 : (s + 1) * 128]
                o = s * 128 - HALF
                return pT_hi[:, o : o + 128]

            def emit_pv(s):
                nc.tensor.matmul(
                    po[s // 2][:, s % 2, :],
                    pv_stationary(s),
                    v_aug[j][:],
                    start=(j == 0 and s % 2 == 0),
                    stop=(j == NKJ - 1),
                    skip_group_check=True,
                )

            ev_ls = []
            ev_ds = []

            def emit_pv_and_evac(s):
                emit_pv(s)
                # On the last key tile, po[s//2]'s final write is matmul
                # s=2b+1 — evacuate that bank immediately so the next
                # chunk's accumulation reuses the banks early. The row-sum
                # column and the 128-wide payload are copied separately:
                # the payload lands contiguous so the divide later runs in
                # the DVE's 2x two-port mode. Payload copies go to ACT
                # (which has slack), row-sum copies to DVE.
                if j == NKJ - 1 and s % 2 == 1:
                    b = s // 2
                    ev_l = el_pool.tile([128, 2, 1], f32, tag=f"evl{b}", name=f"evl{b}")
                    nc.vector.tensor_copy(ev_l[:], po[b][:, :, 0:1])
                    ev_d = ed_pool.tile(
                        [128, 2, HD], f32, tag=f"evd{b}", name=f"evd{b}"
                    )
                    nc.scalar.copy(ev_d[:], po[b][:, :, 1 : HD + 1])
                    ev_ls.append(ev_l)
                    ev_ds.append(ev_d)

            # QK(t+1) is emitted after only TWO of PV(t)'s eight matmuls so
            # the next iteration's scores are ready before the exp engines
            # drain. At a chunk start (j==0) the PV matmuls additionally
            # wait on the previous chunk's PSUM evacuation, so QK goes first.
            pre = 0 if j == 0 else 2
            for s in range(pre):
                emit_pv_and_evac(s)
            if t + 1 < len(iters):
                sT_cur = emit_qk(*iters[t + 1])
            for s in range(pre, NSUB):
                emit_pv_and_evac(s)
            if j == NKJ - 1:
                # Smear the chunk's normalization across the next chunk's
                # iterations: one reciprocal or divide per iteration keeps
                # each per-iteration FIFO injection under ~150ns.
                recs = [None] * (NSUB // 2)
                eh, eci = h, ci

                def mk_recip(b, ev_l=None):
                    def go():
                        rec = r_pool.tile([128, 2, 1], f32, tag="rec", name="rec")
                        nc.vector.reciprocal(rec[:], ev_l[:])
                        recs[b] = rec
                    return go

                def mk_div(s, ev_d=None):
                    def go():
                        sub = s % 2
                        osb = o_pool.tile([128, HD], f32, tag="osb", name="osb")
                        nc.vector.tensor_scalar_mul(
                            osb[:], ev_d[:, sub, :], recs[s // 2][:, sub, :]
                        )
                        r0 = eci * QCH + s * 128
                        nc.sync.dma_start(oo[eh, r0 : r0 + 128, :], osb[:])
                    return go

                for b in range(NSUB // 2):
                    deferred.append((t + 1 + b, mk_recip(b, ev_l=ev_ls[b])))
                for s in range(NSUB):
                    deferred.append((t + 3 + s, mk_div(s, ev_d=ev_ds[s // 2])))
                del po_all[(h, ci)]
            while deferred and deferred[0][0] <= t:
                deferred.pop(0)[1]()
        for _, fn in deferred:
            fn()

    nc.finalize()
    return nc


def _get_bass():
    global _BASS
    if _BASS is None:
        _BASS = _build()
    return _BASS


def _fallback(q, k, v, mask):
    # exact reference math on host, one head at a time (nonzero mask path)
    rep = NH // NKV
    out = np.empty((SEQ, NH, HD), np.float32)
    kh = k.reshape(SEQ, NKV, HD)
    vh = v.reshape(SEQ, NKV, HD)
    for g in range(NH):
        s = (q.reshape(SEQ, NH, HD)[:, g, :] @ kh[:, g // rep, :].T) * np.float32(SCALE)
        s = s + mask
        s -= s.max(axis=-1, keepdims=True)
        p = np.exp(s)
        p /= p.sum(axis=-1, keepdims=True)
        out[:, g, :] = p @ vh[:, g // rep, :]
    return out.reshape(SEQ, NH * HD)


def make_in_maps(q, k, v):
    qh = q.reshape(SEQ, NH, HD)
    kh = k.reshape(SEQ, NKV, HD)
    vh = v.reshape(SEQ, NKV, HD)
    in_maps = []
    for c in range(NCORES):
        qT = np.ascontiguousarray(
            qh[:, HPC * c : HPC * (c + 1), :].transpose(1, 2, 0).astype(np.float16)
        ).reshape(HPC * HD, SEQ)
        kTc = np.ascontiguousarray(kh[:, c, :].T.astype(np.float16))
        vc = np.empty((SEQ, HD + 1), np.float16)
        vc[:, 0] = 1.0
        vc[:, 1:] = vh[:, c, :].astype(np.float16)
        in_maps.append({"qT": qT, "kT": kTc, "v": vc})
    return in_maps


def kernel(q, k, v, mask):
    q = np.ascontiguousarray(np.asarray(q, dtype=np.float32))
    k = np.ascontiguousarray(np.asarray(k, dtype=np.float32))
    v = np.ascontiguousarray(np.asarray(v, dtype=np.float32))
    mask = np.asarray(mask, dtype=np.float32)
    if mask.any():
        return _fallback(q, k, v, mask)

    nc = _get_bass()
    in_maps = make_in_maps(q, k, v)

    from concourse.bass_utils import run_bass_kernel_spmd

    res = run_bass_kernel_spmd(nc, in_maps, list(range(NCORES)))
    out = np.empty((SEQ, NH, HD), np.float32)
    for c in range(NCORES):
        oc = np.asarray(res.results[c]["o"])  # [HPC, SEQ, HD]
        out[:, HPC * c : HPC * (c + 1), :] = oc.transpose(1, 0, 2)
    return out.reshape(SEQ, NH * HD)


# revision 18
# speedup vs baseline: 1.5209x; 1.0494x over previous
"""GQA attention (32 q-heads, 8 kv-heads, d=128, s=2048) on 8 trn2 cores.

Sharding: one kv-head + its 4 q-heads per core (pure head-parallel, no
cross-core communication). The host pre-transposes q/k during sharding so
the device needs no on-chip transposes.

Device algorithm per core (fp16 data path, fp32 accumulation):
  scoresT[kj, qi] = kT_tile.T @ qT          (PE fp16, stationary = kT tile)
  probsT[:, 0:512]   = exp(scoresT * 1/sqrt(d))   (ACT, scale fused, fp16 out)
  probsT[:, 512:1024] = schraudolph(scoresT)      (DVE: int16(s*C1+C2) whose
                                             bit pattern IS fp16 2^(s*SCALE*log2e);
                                             one tensor_scalar, bitcast out)
  out[qi, 0:129] += probsT_tile.T @ [1|v]   (PE fp16; col 0 accumulates the
                                             softmax row-sum, cols 1..128 P@V,
                                             fp32 PSUM accumulation)
  out[qi, d] = out[qi, 1+d] / out[qi, 0]    (split evacuation: row-sum and
                                             payload copied separately so the
                                             divides read contiguous fp32 at
                                             DVE 2x rate; recips + divides
                                             smeared one-per-iteration across
                                             the next chunk to avoid engine
                                             FIFO convoys)

The exp work is split between the scalar engine (exact spline exp, ~1
elem/cycle/lane @1.2GHz) and the vector engine (Schraudolph bitwise exp,
~1 elem/cycle/lane @0.96GHz), which together keep exp off the critical
path; the kernel is then tensor-engine bound (QK + PV streaming).

Schraudolph: int16 z = s_raw*C1 + C2 with C1 = SCALE*2^10/ln2 and
C2 = 15*2^10 + c_corr; bitcast<fp16>(z) = 2^(s_scaled*log2e) with the
mantissa linearly interpolating between exponent steps (max rel err
~4%, zero-mean with c_corr tuned). Queries in columns 512:1024 of each
chunk get the approximate path; measured end-to-end absmax-relative
error ~7.6e-3 vs the fp32 reference (tolerance 2e-2). Saturation-safe:
z stays in [900, 29000] for |scaled scores| < 9.7 (data max ~8.6).

No max-subtraction: scaled scores are < ~9, so exp < e^9 = 8100 fits
fp16 (max 65504) and fp32 PSUM sums comfortably.
The additive mask is all-zeros by construction in this problem; if a
nonzero mask ever shows up we fall back to an exact host computation.
"""

import numpy as np

SEQ = 2048
NH = 32
NKV = 8
HD = 128
HPC = NH // NKV  # q heads per core (= per kv head)
NCORES = 8
SCALE = 1.0 / float(np.sqrt(np.float32(HD)))

# Schraudolph constants (fp16 bit pattern via int16 affine of raw scores)
C1 = SCALE * 1024.0 / float(np.log(2.0))
C2_CORR = -16.0
C2 = 15.0 * 1024.0 + C2_CORR

_BASS = None


def _build():
    from contextlib import ExitStack

    import concourse.tile as tile
    from concourse import bacc, mybir

    f32 = mybir.dt.float32
    f16 = mybir.dt.float16
    i16 = mybir.dt.int16
    # Bacc (not bare Bass): its compile() pass splits >1-wait matmuls via
    # event semaphores, which walrus requires.
    nc = bacc.Bacc(None)
    qT = nc.declare_dram_parameter("qT", [HPC * HD, SEQ], f16, isOutput=False)
    kT = nc.declare_dram_parameter("kT", [HD, SEQ], f16, isOutput=False)
    # v arrives with a leading all-ones column: PV matmuls against [1|v]
    # accumulate the softmax row-sum in output column 0 for free, and a
    # host-built ones column keeps each matmul at <=2 sync waits (the
    # Matmult/LDWEIGHTS wait-slot limit walrus enforces).
    vv = nc.declare_dram_parameter("v", [SEQ, HD + 1], f16, isOutput=False)
    # Output in device-native subtile order [h, chunk, partition, subtile, d]
    # (one contiguous DMA per chunk); the host untangles it with a transpose.
    oo = nc.declare_dram_parameter(
        "o", [HPC, SEQ // 1024, 128, 1024 // 128, HD], f32, isOutput=True
    )

    NKJ = SEQ // 128  # 16 key tiles
    QCH = 1024  # qi chunk: 2 matmul chunks per key tile
    NCHUNK = SEQ // QCH
    NSUB = QCH // 128  # qi sub-tiles (PV accumulator groups) per chunk
    HALF = 512  # ACT/DVE split point within each chunk (multiple of 128).
    # The two halves of each score tile live in SEPARATE PSUM tiles (1 bank
    # each) so the ACT exp and DVE schraudolph have fully independent
    # dependency chains — with a single shared tile, walrus consolidates
    # the QK(t+2) write-after-read wait by chaining sch(t) behind exp(t),
    # serializing the two exp engines.
    EXP = mybir.ActivationFunctionType.Exp
    COPY = mybir.ActivationFunctionType.Copy
    MULT = mybir.AluOpType.mult
    ADD = mybir.AluOpType.add

    with tile.TileContext(nc) as tc, ExitStack() as ctx:
        const = ctx.enter_context(tc.tile_pool(name="const", bufs=1))
        sTl_pool = ctx.enter_context(tc.tile_pool(name="sTl", bufs=2, space="PSUM"))
        sTh_pool = ctx.enter_context(tc.tile_pool(name="sTh", bufs=2, space="PSUM"))
        po_pool = ctx.enter_context(tc.tile_pool(name="po", bufs=1, space="PSUM"))
        pl_pool = ctx.enter_context(tc.tile_pool(name="pLo", bufs=6))
        ph_pool = ctx.enter_context(tc.tile_pool(name="pHi", bufs=6))
        o_pool = ctx.enter_context(tc.tile_pool(name="osb", bufs=2))
        r_pool = ctx.enter_context(tc.tile_pool(name="recip", bufs=8))
        el_pool = ctx.enter_context(tc.tile_pool(name="evacL", bufs=8))
        ed_pool = ctx.enter_context(tc.tile_pool(name="evacD", bufs=8))

        # Preloads are split to slice granularity and emitted in first-use
        # order (DMAs drain roughly in emission order): the first key tile,
        # the first q chunk and the v tiles land first so compute starts
        # immediately; the remaining q chunks stream in ahead of first use.
        qT_sb = [
            const.tile([128, SEQ], f16, tag=f"qT{h}", name=f"qTsb{h}")
            for h in range(HPC)
        ]
        kT_sb = const.tile([128, SEQ], f16, tag="kT")
        v_aug = [
            const.tile([128, HD + 1], f16, tag=f"vaug{j}", name=f"vaug{j}")
            for j in range(NKJ)
        ]

        def load_kt(j):
            nc.sync.dma_start(
                kT_sb[:, j * 128 : (j + 1) * 128], kT[:, j * 128 : (j + 1) * 128]
            )

        def load_qt(h, ci):
            nc.sync.dma_start(
                qT_sb[h][:, ci * QCH : (ci + 1) * QCH],
                qT[h * 128 : (h + 1) * 128, ci * QCH : (ci + 1) * QCH],
            )

        load_kt(0)
        load_qt(0, 0)
        for j in range(NKJ):
            nc.sync.dma_start(v_aug[j][:], vv[j * 128 : (j + 1) * 128, :])
            if j > 0:
                load_kt(j)
        for h in range(HPC):
            for ci in range(NCHUNK):
                if (h, ci) != (0, 0):
                    load_qt(h, ci)

        # Software-pipelined emission over the flat (head, chunk, key-tile)
        # space: QK for iteration t+1 is emitted BEFORE most of PV of
        # iteration t, so the in-order PE stream never sits behind the
        # exp/schraudolph of t+1.
        iters = [
            (h, ci, j)
            for h in range(HPC)
            for ci in range(NCHUNK)
            for j in range(NKJ)
        ]
        po_all = {}
        # Deferred epilogue work, smeared across later iterations so no
        # engine FIFO ever sees a convoy of epilogue instructions that
        # would stall the just-in-time exp -> PV feed. Each entry is
        # (due_t, emit_fn).
        deferred = []

        def emit_qk(h, ci, j):
            sl = sTl_pool.tile([128, HALF], f32, tag="sTl", name="sTl")
            sh = sTh_pool.tile([128, QCH - HALF], f32, tag="sTh", name="sTh")
            q_sl = qT_sb[h][:, ci * QCH : (ci + 1) * QCH]
            kt_sl = kT_sb[:, j * 128 : (j + 1) * 128]
            nc.tensor.matmul(sl[:], kt_sl, q_sl[:, 0:HALF], start=True, stop=True)
            nc.tensor.matmul(sh[:], kt_sl, q_sl[:, HALF:QCH], start=True, stop=True)
            return sl, sh

        def emit_exps(t):
            # exp split: ACT takes qi columns [0:HALF] (exact spline exp),
            # DVE takes [HALF:QCH] (Schraudolph bitwise exp via int16 affine).
            pT_lo = pl_pool.tile([128, HALF], f16, tag="pLo", name="pLo")
            pT_hi = ph_pool.tile([128, QCH - HALF], f16, tag="pHi", name="pHi")
            sl, sh = sT_all.pop(t)
            nc.scalar.activation(pT_lo[:], sl[:], EXP, scale=SCALE)
            nc.vector.tensor_scalar(
                pT_hi[:].bitcast(i16), sh[:], float(C1), float(C2), MULT, ADD
            )
            return pT_lo, pT_hi

        def emit_pvs(t):
            # PV matmuls for iteration t, emitted one iteration late so the
            # exp engines (which must wait for QK(t)) have a full extra
            # iteration of slack before the PE consumes their output — the
            # PE never stalls on a just-in-time probs tile.
            h, ci, j = iters[t]
            if j == 0:
                po_all[(h, ci)] = [
                    po_pool.tile([128, 2, HD + 1], f32, tag=f"po{b}", name=f"po{b}")
                    for b in range(NSUB // 2)
                ]
            po = po_all[(h, ci)]
            pT_lo, pT_hi = pT_all.pop(t)

            def pv_stationary(s):
                if s * 128 < HALF:
                    return pT_lo[:, s * 128 : (s + 1) * 128]
                o = s * 128 - HALF
                return pT_hi[:, o : o + 128]

            for s in range(NSUB):
                # Two PV accumulator groups packed per PSUM bank: the s%2==0
                # group opens with start=True, which clears has_written for
                # the WHOLE bank, so its s%2==1 sibling keeps start=False
                # even on its first matmul (cleared bits make that first
                # write an overwrite, per-element).
                nc.tensor.matmul(
                    po[s // 2][:, s % 2, :],
                    pv_stationary(s),
                    v_aug[j][:],
                    start=(j == 0 and s % 2 == 0),
                    stop=(j == NKJ - 1),
                    skip_group_check=True,
                )
                # On the last key tile, po[s//2]'s final write is matmul
                # s=2b+1 — evacuate that bank immediately so the next
                # chunk's accumulation reuses the banks early. The row-sum
                # column and the 128-wide payload are copied separately:
                # the payload lands contiguous so the divide later runs in
                # the DVE's 2x two-port mode. Payload copies go to ACT
                # (which has slack), row-sum copies to DVE.
                if j == NKJ - 1 and s % 2 == 1:
                    b = s // 2
                    ev_l = el_pool.tile([128, 2, 1], f32, tag=f"evl{b}", name=f"evl{b}")
                    nc.vector.tensor_copy(ev_l[:], po[b][:, :, 0:1])
                    ev_d = ed_pool.tile(
                        [128, 2, HD], f32, tag=f"evd{b}", name=f"evd{b}"
                    )
                    nc.scalar.copy(ev_d[:], po[b][:, :, 1 : HD + 1])
                    ev_ls.append(ev_l)
                    ev_ds.append(ev_d)
            if j == NKJ - 1:
                emit_chunk_epilogue(t, h, ci)
                del po_all[(h, ci)]

        def emit_chunk_epilogue(t, eh, eci):
            # Smear the chunk's normalization across the following
            # iterations: one reciprocal or divide per iteration keeps each
            # per-iteration FIFO injection under ~150ns. One batched DMA
            # per chunk ships the result.
            recs = [None] * (NSUB // 2)
            osb = o_pool.tile([128, NSUB, HD], f32, tag="osb", name="osb")
            my_evl, my_evd = list(ev_ls), list(ev_ds)
            ev_ls.clear()
            ev_ds.clear()

            def mk_recip(b):
                def go():
                    rec = r_pool.tile([128, 2, 1], f32, tag="rec", name="rec")
                    nc.vector.reciprocal(rec[:], my_evl[b][:])
                    recs[b] = rec
                return go

            def mk_div(s):
                def go():
                    nc.vector.tensor_scalar_mul(
                        osb[:, s, :], my_evd[s // 2][:, s % 2, :],
                        recs[s // 2][:, s % 2, :],
                    )
                return go

            def mk_dma():
                def go():
                    nc.sync.dma_start(oo[eh, eci], osb[:])
                return go

            for b in range(NSUB // 2):
                deferred.append((t + 2 + b, mk_recip(b)))
            for s in range(NSUB):
                deferred.append((t + 4 + s, mk_div(s)))
            deferred.append((t + 4 + NSUB, mk_dma()))

        sT_all = {}
        pT_all = {}
        ev_ls = []
        ev_ds = []
        sT_all[0] = emit_qk(*iters[0])
        for t, (h, ci, j) in enumerate(iters):
            pT_all[t] = emit_exps(t)
            if t + 1 < len(iters):
                sT_all[t + 1] = emit_qk(*iters[t + 1])
            if t > 0:
                emit_pvs(t - 1)
            while deferred and deferred[0][0] <= t:
                deferred.pop(0)[1]()
        emit_pvs(len(iters) - 1)
        for _, fn in deferred:
            fn()

    nc.finalize()
    return nc


def _get_bass():
    global _BASS
    if _BASS is None:
        _BASS = _build()
    return _BASS


def _fallback(q, k, v, mask):
    # exact reference math on host, one head at a time (nonzero mask path)
    rep = NH // NKV
    out = np.empty((SEQ, NH, HD), np.float32)
    kh = k.reshape(SEQ, NKV, HD)
    vh = v.reshape(SEQ, NKV, HD)
    for g in range(NH):
        s = (q.reshape(SEQ, NH, HD)[:, g, :] @ kh[:, g // rep, :].T) * np.float32(SCALE)
        s = s + mask
        s -= s.max(axis=-1, keepdims=True)
        p = np.exp(s)
        p /= p.sum(axis=-1, keepdims=True)
        out[:, g, :] = p @ vh[:, g // rep, :]
    return out.reshape(SEQ, NH * HD)


def make_in_maps(q, k, v):
    qh = q.reshape(SEQ, NH, HD)
    kh = k.reshape(SEQ, NKV, HD)
    vh = v.reshape(SEQ, NKV, HD)
    in_maps = []
    for c in range(NCORES):
        qT = np.ascontiguousarray(
            qh[:, HPC * c : HPC * (c + 1), :].transpose(1, 2, 0).astype(np.float16)
        ).reshape(HPC * HD, SEQ)
        kTc = np.ascontiguousarray(kh[:, c, :].T.astype(np.float16))
        vc = np.empty((SEQ, HD + 1), np.float16)
        vc[:, 0] = 1.0
        vc[:, 1:] = vh[:, c, :].astype(np.float16)
        in_maps.append({"qT": qT, "kT": kTc, "v": vc})
    return in_maps


def kernel(q, k, v, mask):
    q = np.ascontiguousarray(np.asarray(q, dtype=np.float32))
    k = np.ascontiguousarray(np.asarray(k, dtype=np.float32))
    v = np.ascontiguousarray(np.asarray(v, dtype=np.float32))
    mask = np.asarray(mask, dtype=np.float32)
    if mask.any():
        return _fallback(q, k, v, mask)

    nc = _get_bass()
    in_maps = make_in_maps(q, k, v)

    from concourse.bass_utils import run_bass_kernel_spmd

    res = run_bass_kernel_spmd(nc, in_maps, list(range(NCORES)))
    out = np.empty((SEQ, NH, HD), np.float32)
    for c in range(NCORES):
        # [HPC, NCHUNK, 128, NSUB, HD] in device subtile order; qi is
        # (chunk, subtile, partition).
        oc = np.asarray(res.results[c]["o"])
        oc = oc.transpose(1, 3, 2, 0, 4).reshape(SEQ, HPC, HD)
        out[:, HPC * c : HPC * (c + 1), :] = oc
    return out.reshape(SEQ, NH * HD)
